# revision 9
# baseline (speedup 1.0000x reference)
"""AttentionReadout kernel for 8 Trainium2 NeuronCores.

Math (per graph g): pooled[g] = sum_i attn_i * x_i with
  attn_i = e_i / sum_{j in g} e_j,  e_i = exp(tanh(x_i @ W1 + b1) @ W2 + b2)
  out = pooled @ Wt + bt

Sharding: graph-aligned data parallel. Core k owns graphs [128k, 128k+128)
and exactly the (contiguous, since batch is sorted) nodes of those graphs.
Each core computes its own 128 graphs end-to-end; no collectives. Host
concatenates the 8 [128, 128] output shards.

Device pipeline (software-pipelined, lag 2 between stages so the PE always
has ready work):
  stage A (macro m):  DMA x natural [n,c] + DMA-transpose [c,n] (bf16),
                      PE: hT[64,512] = W1b.T @ xT;  ACT: h = tanh(hT+b1)
  stage B (macro m-1): PE per 128-chunk: scores[n,1] = h_chunk.T @ W2;
                      ACT: e[128,4] = exp(scores + b2) (bf16)
  stage C (macro m-2): DVE/GpSimd per chunk: ohe[n,g] = (iota==gid)*e;
                      PE per chunk: pacc[g,0:129] += ohe.T @ [x | 1]
                      (column 128 of the rhs is constant 1 -> accumulates
                      the softmax denominator for free)
Final: den=pacc[:,128]; pooled_n = pacc[:,0:128]/den (DVE); PE transpose;
outT[o,g] = Wt.T @ pooled_n.T + bt; DMA out.
"""

import numpy as np
import ml_dtypes
from contextlib import ExitStack

import concourse.bass as bass
import concourse.bacc as bacc
import concourse.tile as tile
from concourse import mybir
from concourse.bass_utils import run_bass_kernel_spmd

N_CORES = 8
G = 1024
GPC = G // N_CORES  # 128 graphs per core
IN_C = 128
HID = 64
OUT_C = 128
MACRO = 512          # nodes per macro tile
DMAT = 2048          # nodes per natural-load DMA tile
TPOSE = 2048         # nodes per DMA-transpose tile
CHUNK = 128          # nodes per chunk (PE contraction width)
JPM = MACRO // CHUNK  # chunks per macro
BF16 = mybir.dt.bfloat16
F32 = mybir.dt.float32

_CACHE = {}


def _build(npad):
    nm = npad // CHUNK        # gid columns
    n_macros = npad // MACRO

    nc = bacc.Bacc("TRN2", target_bir_lowering=False, debug=False,
                   num_devices=N_CORES)

    x_nat = nc.dram_tensor("x_nat", [npad, IN_C], BF16, kind="ExternalInput").ap()
    gid_d = nc.dram_tensor("gid", [CHUNK, nm], F32, kind="ExternalInput").ap()
    iota_d = nc.dram_tensor("iota", [CHUNK, GPC], BF16, kind="ExternalInput").ap()
    w1_d = nc.dram_tensor("w1", [IN_C, HID], BF16, kind="ExternalInput").ap()
    b1_d = nc.dram_tensor("b1", [HID, 1], F32, kind="ExternalInput").ap()
    w2_d = nc.dram_tensor("w2", [HID, 1], BF16, kind="ExternalInput").ap()
    b2_d = nc.dram_tensor("b2", [CHUNK, 1], F32, kind="ExternalInput").ap()
    wt_d = nc.dram_tensor("wt", [IN_C, OUT_C], F32, kind="ExternalInput").ap()
    bt_d = nc.dram_tensor("bt", [OUT_C, 1], F32, kind="ExternalInput").ap()
    id_d = nc.dram_tensor("idm", [128, 128], F32, kind="ExternalInput").ap()
    out_d = nc.dram_tensor("outT", [OUT_C, GPC], F32, kind="ExternalOutput").ap()

    with tile.TileContext(nc) as tc, ExitStack() as ctx:
        consts = ctx.enter_context(tc.tile_pool(name="consts", bufs=1))
        xn_pool = ctx.enter_context(tc.tile_pool(name="xn", bufs=4))
        xt_pool = ctx.enter_context(tc.tile_pool(name="xt", bufs=4))
        hb_pool = ctx.enter_context(tc.tile_pool(name="hb", bufs=4))
        e4_pool = ctx.enter_context(tc.tile_pool(name="e4", bufs=6))
        ohe_pool = ctx.enter_context(tc.tile_pool(name="ohe", bufs=12))
        fin_pool = ctx.enter_context(tc.tile_pool(name="fin", bufs=1))
        ph_pool = ctx.enter_context(tc.tile_pool(name="ph", bufs=3, space="PSUM"))
        ps_pool = ctx.enter_context(tc.tile_pool(name="ps", bufs=2, space="PSUM"))
        pacc_pool = ctx.enter_context(tc.tile_pool(name="pacc", bufs=1, space="PSUM"))
        pfin_pool = ctx.enter_context(tc.tile_pool(name="pfin", bufs=1, space="PSUM"))

        # constants
        gid_s = consts.tile([CHUNK, nm], F32)
        nc.sync.dma_start(gid_s[:], gid_d[:])
        iota_s = consts.tile([CHUNK, GPC], BF16)
        nc.sync.dma_start(iota_s[:], iota_d[:])
        w1_s = consts.tile([IN_C, HID], BF16)
        nc.sync.dma_start(w1_s[:], w1_d[:])
        b1_s = consts.tile([HID, 1], F32)
        nc.sync.dma_start(b1_s[:], b1_d[:])
        w2_s = consts.tile([HID, 1], BF16)
        nc.sync.dma_start(w2_s[:], w2_d[:])
        b2_s = consts.tile([CHUNK, 1], F32)
        nc.sync.dma_start(b2_s[:], b2_d[:])
        wt_s = consts.tile([IN_C, OUT_C], F32)
        nc.sync.dma_start(wt_s[:], wt_d[:])
        bt_s = consts.tile([OUT_C, 1], F32)
        nc.sync.dma_start(bt_s[:], bt_d[:])
        id_s = consts.tile([128, 128], F32)
        nc.sync.dma_start(id_s[:], id_d[:])

        pacc = pacc_pool.tile([GPC, IN_C + 1], F32)  # [g, c | den]

        n_chunks_total = n_macros * JPM
        tiles = {}  # m -> dict of stage tiles
        state = {"xn": None, "xt": None, "ci": 0}

        def stage_a(m):
            n0 = m * MACRO
            if n0 % DMAT == 0:
                xn = xn_pool.tile([CHUNK, DMAT // CHUNK, IN_C + 1], BF16)
                nc.sync.dma_start(
                    xn[:, :, 0:IN_C],
                    x_nat[n0:n0 + DMAT, :].rearrange("(j p) c -> p j c", p=CHUNK),
                )
                nc.gpsimd.memset(xn[:, :, IN_C:IN_C + 1], 1.0)
                state["xn"] = xn
            if n0 % TPOSE == 0:
                xt = xt_pool.tile([IN_C, TPOSE], BF16)
                nc.sync.dma_start(xt[:], x_nat[n0:n0 + TPOSE, :], transpose=True)
                state["xt"] = xt
            mt = n0 % TPOSE
            ph = ph_pool.tile([HID, MACRO], F32)
            nc.tensor.matmul(ph[:], w1_s[:], state["xt"][:, mt:mt + MACRO],
                             start=True, stop=True)
            hb = hb_pool.tile([HID, MACRO], BF16)
            nc.scalar.activation(hb[:], ph[:],
                                 mybir.ActivationFunctionType.Tanh, bias=b1_s[:])
            tiles[m] = {"hb": hb, "xn": state["xn"], "mj": (n0 % DMAT) // MACRO}

        def stage_b(m):
            t = tiles[m]
            ps = ps_pool.tile([CHUNK, JPM], F32)
            for j in range(JPM):
                nc.tensor.matmul(ps[:, j:j + 1],
                                 t["hb"][:, j * CHUNK:(j + 1) * CHUNK], w2_s[:],
                                 start=True, stop=True)
            e4 = e4_pool.tile([CHUNK, JPM], F32)
            nc.scalar.activation(e4[:], ps[:],
                                 mybir.ActivationFunctionType.Exp, bias=b2_s[:])
            t["e4"] = e4

        def stage_c1(m):
            t = tiles[m]
            ohes = []
            for j in range(JPM):
                q = m * JPM + j
                ohe = ohe_pool.tile([CHUNK, GPC], BF16)
                nc.vector.tensor_scalar(
                    ohe[:], iota_s[:],
                    gid_s[:, q:q + 1], t["e4"][:, j:j + 1],
                    mybir.AluOpType.is_equal, mybir.AluOpType.mult)
                ohes.append(ohe)
            t["ohes"] = ohes

        def stage_c2(m):
            t = tiles[m]
            for j in range(JPM):
                ci = state["ci"]
                nc.tensor.matmul(pacc[:], t["ohes"][j][:],
                                 t["xn"][:, t["mj"] * JPM + j, :],
                                 start=(ci == 0), stop=(ci == n_chunks_total - 1))
                state["ci"] = ci + 1
            del tiles[m]

        for m in range(n_macros + 3):
            if m < n_macros:
                stage_a(m)
            if 1 <= m <= n_macros:
                stage_b(m - 1)
            if 2 <= m <= n_macros + 1:
                stage_c1(m - 2)
            if m >= 3:
                stage_c2(m - 3)

        # ---- final: normalize, transform, write out ----
        rden = fin_pool.tile([GPC, 1], F32, tag="rden")
        nc.vector.reciprocal(rden[:], pacc[:, IN_C:IN_C + 1])
        pooln = fin_pool.tile([GPC, IN_C], F32, tag="pooln")
        nc.vector.tensor_scalar(pooln[:], pacc[:, 0:IN_C], rden[:], None,
                                mybir.AluOpType.mult)
        ptr = pfin_pool.tile([IN_C, GPC], F32, tag="ptr")
        nc.tensor.transpose(ptr[:], pooln[:], id_s[:])
        poolT = fin_pool.tile([IN_C, GPC], F32, tag="poolT")
        nc.scalar.copy(poolT[:], ptr[:])
        pfin = pfin_pool.tile([OUT_C, GPC], F32, tag="pfin")
        nc.tensor.matmul(pfin[:], wt_s[:], poolT[:], start=True, stop=True)
        outT_s = fin_pool.tile([OUT_C, GPC], F32, tag="outT")
        nc.scalar.activation(outT_s[:], pfin[:],
                             mybir.ActivationFunctionType.Identity, bias=bt_s[:])
        nc.sync.dma_start(out_d[:], outT_s[:])

    nc.compile()
    return nc


def kernel(x, batch, W1, b1, W2, b2, Wt, bt, _trace=False, _trace_kwargs=None):
    x = np.asarray(x)
    batch = np.asarray(batch)
    W1 = np.asarray(W1, dtype=np.float32)
    b1 = np.asarray(b1, dtype=np.float32)
    W2 = np.asarray(W2, dtype=np.float32)
    b2 = np.asarray(b2, dtype=np.float32)
    Wt = np.asarray(Wt, dtype=np.float32)
    bt = np.asarray(bt, dtype=np.float32)

    starts = np.searchsorted(batch, np.arange(N_CORES + 1) * GPC).astype(np.int64)
    counts = np.diff(starts)
    npad = int(-(-counts.max() // DMAT) * DMAT)
    nm = npad // CHUNK

    key = npad
    if key not in _CACHE:
        _CACHE[key] = _build(npad)
    nc = _CACHE[key]

    bf16 = ml_dtypes.bfloat16
    iota = np.broadcast_to(np.arange(GPC, dtype=np.float32), (CHUNK, GPC))
    common = {
        "iota": iota.astype(bf16),
        "w1": W1.astype(bf16),
        "b1": b1.reshape(HID, 1).astype(np.float32),
        "w2": W2.reshape(HID, 1).astype(bf16),
        "b2": np.full((CHUNK, 1), float(b2.ravel()[0]), dtype=np.float32),
        "wt": Wt.astype(np.float32),
        "bt": bt.reshape(OUT_C, 1).astype(np.float32),
        "idm": np.eye(128, dtype=np.float32),
    }
    in_maps = []
    for k in range(N_CORES):
        s, e = int(starts[k]), int(starts[k + 1])
        cnt = e - s
        x_nat = np.zeros((npad, IN_C), dtype=bf16)
        x_nat[:cnt] = x[s:e].astype(bf16)
        gid_lin = np.full(npad, -1.0, dtype=np.float32)
        gid_lin[:cnt] = (batch[s:e] - k * GPC).astype(np.float32)
        gid = np.ascontiguousarray(gid_lin.reshape(nm, CHUNK).T)
        in_maps.append({"x_nat": x_nat, "gid": gid, **common})

    res = run_bass_kernel_spmd(
        nc, in_maps, core_ids=list(range(N_CORES)),
        trace=_trace, **(_trace_kwargs or {}))

    out = np.empty((G, OUT_C), dtype=np.float32)
    for k in range(N_CORES):
        out[k * GPC:(k + 1) * GPC, :] = res.results[k]["outT"].T
    if _trace:
        return out, res
    return out


# revision 10
# speedup vs baseline: 1.0551x; 1.0551x over previous
"""AttentionReadout kernel for 8 Trainium2 NeuronCores.

Math (per graph g): pooled[g] = sum_i attn_i * x_i with
  attn_i = e_i / sum_{j in g} e_j,  e_i = exp(tanh(x_i @ W1 + b1) @ W2 + b2)
  out = pooled @ Wt + bt

Sharding: graph-aligned data parallel. Core k owns graphs [128k, 128k+128)
and exactly the (contiguous, since batch is sorted) nodes of those graphs.
Each core computes its own 128 graphs end-to-end; no collectives. Host
concatenates the 8 [128, 128] output shards.

Device pipeline (software-pipelined, lag 2 between stages so the PE always
has ready work):
  stage A (macro m):  DMA x natural [n,c] + DMA-transpose [c,n] (bf16),
                      PE: hT[64,512] = W1b.T @ xT;  ACT: h = tanh(hT+b1)
  stage B (macro m-1): PE per 128-chunk: scores[n,1] = h_chunk.T @ W2;
                      ACT: e[128,4] = exp(scores + b2) (bf16)
  stage C (macro m-2): DVE/GpSimd per chunk: ohe[n,g] = (iota==gid)*e;
                      PE per chunk: pacc[g,0:129] += ohe.T @ [x | 1]
                      (column 128 of the rhs is constant 1 -> accumulates
                      the softmax denominator for free)
Final: den=pacc[:,128]; pooled_n = pacc[:,0:128]/den (DVE); PE transpose;
outT[o,g] = Wt.T @ pooled_n.T + bt; DMA out.
"""

import numpy as np
import ml_dtypes
from contextlib import ExitStack

import concourse.bass as bass
import concourse.bacc as bacc
import concourse.tile as tile
from concourse import mybir
from concourse.bass_utils import run_bass_kernel_spmd

N_CORES = 8
G = 1024
GPC = G // N_CORES  # 128 graphs per core
IN_C = 128
HID = 64
OUT_C = 128
MACRO = 512          # nodes per macro tile
DMAT = 2048          # nodes per natural-load DMA tile
TPOSE = 2048         # nodes per DMA-transpose tile
CHUNK = 128          # nodes per chunk (PE contraction width)
JPM = MACRO // CHUNK  # chunks per macro
BF16 = mybir.dt.bfloat16
F32 = mybir.dt.float32

_CACHE = {}


def _build(npad):
    nm = npad // CHUNK        # gid columns
    n_macros = npad // MACRO

    nc = bacc.Bacc("TRN2", target_bir_lowering=False, debug=False,
                   num_devices=N_CORES)

    x_nat = nc.dram_tensor("x_nat", [npad, IN_C], BF16, kind="ExternalInput").ap()
    gid_d = nc.dram_tensor("gid", [CHUNK, nm], F32, kind="ExternalInput").ap()
    iota_d = nc.dram_tensor("iota", [CHUNK, GPC], BF16, kind="ExternalInput").ap()
    w1_d = nc.dram_tensor("w1", [IN_C, HID], BF16, kind="ExternalInput").ap()
    b1_d = nc.dram_tensor("b1", [HID, 1], F32, kind="ExternalInput").ap()
    w2_d = nc.dram_tensor("w2", [HID, 1], BF16, kind="ExternalInput").ap()
    b2_d = nc.dram_tensor("b2", [CHUNK, 1], F32, kind="ExternalInput").ap()
    wt_d = nc.dram_tensor("wt", [IN_C, OUT_C], F32, kind="ExternalInput").ap()
    bt_d = nc.dram_tensor("bt", [OUT_C, 1], F32, kind="ExternalInput").ap()
    id_d = nc.dram_tensor("idm", [128, 128], F32, kind="ExternalInput").ap()
    out_d = nc.dram_tensor("outT", [OUT_C, GPC], F32, kind="ExternalOutput").ap()

    with tile.TileContext(nc) as tc, ExitStack() as ctx:
        consts = ctx.enter_context(tc.tile_pool(name="consts", bufs=1))
        xn_pool = ctx.enter_context(tc.tile_pool(name="xn", bufs=4))
        xt_pool = ctx.enter_context(tc.tile_pool(name="xt", bufs=4))
        hb_pool = ctx.enter_context(tc.tile_pool(name="hb", bufs=4))
        e4_pool = ctx.enter_context(tc.tile_pool(name="e4", bufs=6))
        ohe_pool = ctx.enter_context(tc.tile_pool(name="ohe", bufs=12))
        fin_pool = ctx.enter_context(tc.tile_pool(name="fin", bufs=1))
        ph_pool = ctx.enter_context(tc.tile_pool(name="ph", bufs=3, space="PSUM"))
        ps_pool = ctx.enter_context(tc.tile_pool(name="ps", bufs=2, space="PSUM"))
        pacc_pool = ctx.enter_context(tc.tile_pool(name="pacc", bufs=1, space="PSUM"))
        pfin_pool = ctx.enter_context(tc.tile_pool(name="pfin", bufs=1, space="PSUM"))

        # constants
        gid_s = consts.tile([CHUNK, nm], F32)
        nc.sync.dma_start(gid_s[:], gid_d[:])
        iota_s = consts.tile([CHUNK, GPC], BF16)
        nc.sync.dma_start(iota_s[:], iota_d[:])
        w1_s = consts.tile([IN_C, HID], BF16)
        nc.sync.dma_start(w1_s[:], w1_d[:])
        b1_s = consts.tile([HID, 1], F32)
        nc.sync.dma_start(b1_s[:], b1_d[:])
        w2_s = consts.tile([HID, 1], BF16)
        nc.sync.dma_start(w2_s[:], w2_d[:])
        b2_s = consts.tile([CHUNK, 1], F32)
        nc.sync.dma_start(b2_s[:], b2_d[:])
        wt_s = consts.tile([IN_C, OUT_C], F32)
        nc.sync.dma_start(wt_s[:], wt_d[:])
        bt_s = consts.tile([OUT_C, 1], F32)
        nc.sync.dma_start(bt_s[:], bt_d[:])
        id_s = consts.tile([128, 128], F32)
        nc.sync.dma_start(id_s[:], id_d[:])

        pacc = pacc_pool.tile([GPC, IN_C + 1], F32)  # [g, c | den]

        n_chunks_total = n_macros * JPM
        tiles = {}  # m -> dict of stage tiles
        state = {"xn": None, "xt": None, "ci": 0}

        def stage_a_dma(m):
            n0 = m * MACRO
            if n0 % DMAT == 0:
                xn = xn_pool.tile([CHUNK, DMAT // CHUNK, IN_C + 1], BF16)
                nc.sync.dma_start(
                    xn[:, :, 0:IN_C],
                    x_nat[n0:n0 + DMAT, :].rearrange("(j p) c -> p j c", p=CHUNK),
                )
                nc.gpsimd.memset(xn[:, :, IN_C:IN_C + 1], 1.0)
                state["xn"] = xn
            if n0 % TPOSE == 0:
                xt = xt_pool.tile([IN_C, TPOSE], BF16)
                nc.sync.dma_start(xt[:], x_nat[n0:n0 + TPOSE, :], transpose=True)
                state["xt"] = xt
            tiles[m] = {"xn": state["xn"], "xt": state["xt"],
                        "mt": n0 % TPOSE, "mj": (n0 % DMAT) // MACRO}

        def stage_a_mm(m):
            t = tiles[m]
            ph = ph_pool.tile([HID, MACRO], F32)
            nc.tensor.matmul(ph[:], w1_s[:], t["xt"][:, t["mt"]:t["mt"] + MACRO],
                             start=True, stop=True)
            hb = hb_pool.tile([HID, MACRO], BF16)
            nc.scalar.activation(hb[:], ph[:],
                                 mybir.ActivationFunctionType.Tanh, bias=b1_s[:])
            t["hb"] = hb

        def stage_b(m):
            t = tiles[m]
            ps = ps_pool.tile([CHUNK, JPM], F32)
            for j in range(JPM):
                nc.tensor.matmul(ps[:, j:j + 1],
                                 t["hb"][:, j * CHUNK:(j + 1) * CHUNK], w2_s[:],
                                 start=True, stop=True)
            e4 = e4_pool.tile([CHUNK, JPM], F32)
            nc.scalar.activation(e4[:], ps[:],
                                 mybir.ActivationFunctionType.Exp, bias=b2_s[:])
            t["e4"] = e4

        def stage_c1(m):
            t = tiles[m]
            ohes = []
            for j in range(JPM):
                q = m * JPM + j
                ohe = ohe_pool.tile([CHUNK, GPC], BF16)
                nc.vector.tensor_scalar(
                    ohe[:], iota_s[:],
                    gid_s[:, q:q + 1], t["e4"][:, j:j + 1],
                    mybir.AluOpType.is_equal, mybir.AluOpType.mult)
                ohes.append(ohe)
            t["ohes"] = ohes

        def stage_c2(m):
            t = tiles[m]
            for j in range(JPM):
                ci = state["ci"]
                nc.tensor.matmul(pacc[:], t["ohes"][j][:],
                                 t["xn"][:, t["mj"] * JPM + j, :],
                                 start=(ci == 0), stop=(ci == n_chunks_total - 1))
                state["ci"] = ci + 1
            del tiles[m]

        for m in range(n_macros + 3):
            if m < n_macros:
                stage_a_dma(m)
            if 1 <= m <= n_macros:
                stage_b(m - 1)
            if 2 <= m <= n_macros + 1:
                stage_c1(m - 2)
            if m >= 3:
                stage_c2(m - 3)
            # mlp last: it depends on the freshest DMA; keeping it behind
            # ready work prevents it from blocking the in-order PE queue
            if m < n_macros:
                stage_a_mm(m)

        # ---- final: normalize, transform, write out ----
        rden = fin_pool.tile([GPC, 1], F32, tag="rden")
        nc.vector.reciprocal(rden[:], pacc[:, IN_C:IN_C + 1])
        pooln = fin_pool.tile([GPC, IN_C], F32, tag="pooln")
        nc.vector.tensor_scalar(pooln[:], pacc[:, 0:IN_C], rden[:], None,
                                mybir.AluOpType.mult)
        ptr = pfin_pool.tile([IN_C, GPC], F32, tag="ptr")
        nc.tensor.transpose(ptr[:], pooln[:], id_s[:])
        poolT = fin_pool.tile([IN_C, GPC], F32, tag="poolT")
        nc.scalar.copy(poolT[:], ptr[:])
        pfin = pfin_pool.tile([OUT_C, GPC], F32, tag="pfin")
        nc.tensor.matmul(pfin[:], wt_s[:], poolT[:], start=True, stop=True)
        outT_s = fin_pool.tile([OUT_C, GPC], F32, tag="outT")
        nc.scalar.activation(outT_s[:], pfin[:],
                             mybir.ActivationFunctionType.Identity, bias=bt_s[:])
        nc.sync.dma_start(out_d[:], outT_s[:])

    nc.compile()
    return nc


def kernel(x, batch, W1, b1, W2, b2, Wt, bt, _trace=False, _trace_kwargs=None):
    x = np.asarray(x)
    batch = np.asarray(batch)
    W1 = np.asarray(W1, dtype=np.float32)
    b1 = np.asarray(b1, dtype=np.float32)
    W2 = np.asarray(W2, dtype=np.float32)
    b2 = np.asarray(b2, dtype=np.float32)
    Wt = np.asarray(Wt, dtype=np.float32)
    bt = np.asarray(bt, dtype=np.float32)

    starts = np.searchsorted(batch, np.arange(N_CORES + 1) * GPC).astype(np.int64)
    counts = np.diff(starts)
    npad = int(-(-counts.max() // DMAT) * DMAT)
    nm = npad // CHUNK

    key = npad
    if key not in _CACHE:
        _CACHE[key] = _build(npad)
    nc = _CACHE[key]

    bf16 = ml_dtypes.bfloat16
    iota = np.broadcast_to(np.arange(GPC, dtype=np.float32), (CHUNK, GPC))
    common = {
        "iota": iota.astype(bf16),
        "w1": W1.astype(bf16),
        "b1": b1.reshape(HID, 1).astype(np.float32),
        "w2": W2.reshape(HID, 1).astype(bf16),
        "b2": np.full((CHUNK, 1), float(b2.ravel()[0]), dtype=np.float32),
        "wt": Wt.astype(np.float32),
        "bt": bt.reshape(OUT_C, 1).astype(np.float32),
        "idm": np.eye(128, dtype=np.float32),
    }
    in_maps = []
    for k in range(N_CORES):
        s, e = int(starts[k]), int(starts[k + 1])
        cnt = e - s
        x_nat = np.zeros((npad, IN_C), dtype=bf16)
        x_nat[:cnt] = x[s:e].astype(bf16)
        gid_lin = np.full(npad, -1.0, dtype=np.float32)
        gid_lin[:cnt] = (batch[s:e] - k * GPC).astype(np.float32)
        gid = np.ascontiguousarray(gid_lin.reshape(nm, CHUNK).T)
        in_maps.append({"x_nat": x_nat, "gid": gid, **common})

    res = run_bass_kernel_spmd(
        nc, in_maps, core_ids=list(range(N_CORES)),
        trace=_trace, **(_trace_kwargs or {}))

    out = np.empty((G, OUT_C), dtype=np.float32)
    for k in range(N_CORES):
        out[k * GPC:(k + 1) * GPC, :] = res.results[k]["outT"].T
    if _trace:
        return out, res
    return out


# revision 12
# speedup vs baseline: 1.2791x; 1.2123x over previous
"""AttentionReadout kernel for 8 Trainium2 NeuronCores.

Math (per graph g): pooled[g] = sum_i attn_i * x_i with
  attn_i = e_i / sum_{j in g} e_j,  e_i = exp(tanh(x_i @ W1 + b1) @ W2 + b2)
  out = pooled @ Wt + bt

Sharding: graph-aligned data parallel. Core k owns graphs [128k, 128k+128)
and exactly the (contiguous, since batch is sorted) nodes of those graphs.
Each core computes its own 128 graphs end-to-end; no collectives. Host
concatenates the 8 [128, 128] output shards.

Device pipeline (software-pipelined, lag 2 between stages so the PE always
has ready work):
  stage A (macro m):  DMA x natural [n,c] + DMA-transpose [c,n] (bf16),
                      PE: hT[64,512] = W1b.T @ xT;  ACT: h = tanh(hT+b1)
  stage B (macro m-1): PE per 128-chunk: scores[n,1] = h_chunk.T @ W2;
                      ACT: e[128,4] = exp(scores + b2) (bf16)
  stage C (macro m-2): DVE/GpSimd per chunk: ohe[n,g] = (iota==gid)*e;
                      PE per chunk: pacc[g,0:129] += ohe.T @ [x | 1]
                      (column 128 of the rhs is constant 1 -> accumulates
                      the softmax denominator for free)
Final: den=pacc[:,128]; pooled_n = pacc[:,0:128]/den (DVE); PE transpose;
outT[o,g] = Wt.T @ pooled_n.T + bt; DMA out.
"""

import numpy as np
import ml_dtypes
from contextlib import ExitStack

import concourse.bass as bass
import concourse.bacc as bacc
import concourse.tile as tile
from concourse import mybir
from concourse.bass_utils import run_bass_kernel_spmd

N_CORES = 8
G = 1024
GPC = G // N_CORES  # 128 graphs per core
IN_C = 128
HID = 64
OUT_C = 128
MACRO = 512          # nodes per macro tile
DMAT = 2048          # nodes per natural-load DMA tile
TPOSE = 2048         # nodes per DMA-transpose tile
CHUNK = 128          # nodes per chunk (PE contraction width)
JPM = MACRO // CHUNK  # chunks per macro
BF16 = mybir.dt.bfloat16
F32 = mybir.dt.float32

_CACHE = {}


def _build(npad):
    nm = npad // CHUNK        # gid columns
    n_macros = npad // MACRO

    nc = bacc.Bacc("TRN2", target_bir_lowering=False, debug=False,
                   num_devices=N_CORES)

    x_nat = nc.dram_tensor("x_nat", [npad, IN_C], BF16, kind="ExternalInput").ap()
    xT_d = nc.dram_tensor("xT", [IN_C, npad], BF16, kind="ExternalInput").ap()
    gid_d = nc.dram_tensor("gid", [CHUNK, nm], F32, kind="ExternalInput").ap()
    iota_d = nc.dram_tensor("iota", [CHUNK, GPC], BF16, kind="ExternalInput").ap()
    w1_d = nc.dram_tensor("w1", [IN_C, HID], BF16, kind="ExternalInput").ap()
    b1_d = nc.dram_tensor("b1", [HID, 1], F32, kind="ExternalInput").ap()
    w2_d = nc.dram_tensor("w2", [HID, 1], BF16, kind="ExternalInput").ap()
    b2_d = nc.dram_tensor("b2", [CHUNK, 1], F32, kind="ExternalInput").ap()
    wt_d = nc.dram_tensor("wt", [IN_C, OUT_C], F32, kind="ExternalInput").ap()
    bt_d = nc.dram_tensor("bt", [OUT_C, 1], F32, kind="ExternalInput").ap()
    id_d = nc.dram_tensor("idm", [128, 128], F32, kind="ExternalInput").ap()
    out_d = nc.dram_tensor("outT", [OUT_C, GPC], F32, kind="ExternalOutput").ap()

    with tile.TileContext(nc) as tc, ExitStack() as ctx:
        consts = ctx.enter_context(tc.tile_pool(name="consts", bufs=1))
        xn_pool = ctx.enter_context(tc.tile_pool(name="xn", bufs=4))
        xt_pool = ctx.enter_context(tc.tile_pool(name="xt", bufs=4))
        hb_pool = ctx.enter_context(tc.tile_pool(name="hb", bufs=4))
        e4_pool = ctx.enter_context(tc.tile_pool(name="e4", bufs=6))
        ohe_pool = ctx.enter_context(tc.tile_pool(name="ohe", bufs=12))
        fin_pool = ctx.enter_context(tc.tile_pool(name="fin", bufs=1))
        ph_pool = ctx.enter_context(tc.tile_pool(name="ph", bufs=3, space="PSUM"))
        ps_pool = ctx.enter_context(tc.tile_pool(name="ps", bufs=2, space="PSUM"))
        pacc_pool = ctx.enter_context(tc.tile_pool(name="pacc", bufs=1, space="PSUM"))
        pfin_pool = ctx.enter_context(tc.tile_pool(name="pfin", bufs=1, space="PSUM"))

        # constants
        gid_s = consts.tile([CHUNK, nm], F32)
        nc.sync.dma_start(gid_s[:], gid_d[:])
        iota_s = consts.tile([CHUNK, GPC], BF16)
        nc.sync.dma_start(iota_s[:], iota_d[:])
        w1_s = consts.tile([IN_C, HID], BF16)
        nc.sync.dma_start(w1_s[:], w1_d[:])
        b1_s = consts.tile([HID, 1], F32)
        nc.sync.dma_start(b1_s[:], b1_d[:])
        w2_s = consts.tile([HID, 1], BF16)
        nc.sync.dma_start(w2_s[:], w2_d[:])
        b2_s = consts.tile([CHUNK, 1], F32)
        nc.sync.dma_start(b2_s[:], b2_d[:])
        wt_s = consts.tile([IN_C, OUT_C], F32)
        nc.sync.dma_start(wt_s[:], wt_d[:])
        bt_s = consts.tile([OUT_C, 1], F32)
        nc.sync.dma_start(bt_s[:], bt_d[:])
        id_s = consts.tile([128, 128], F32)
        nc.sync.dma_start(id_s[:], id_d[:])

        pacc = pacc_pool.tile([GPC, IN_C + 1], F32)  # [g, c | den]

        n_chunks_total = n_macros * JPM
        tiles = {}  # m -> dict of stage tiles
        state = {"xn": None, "xt": None, "ci": 0}

        def stage_a_dma(m):
            n0 = m * MACRO
            if n0 % DMAT == 0:
                xn = xn_pool.tile([CHUNK, DMAT // MACRO, 4, IN_C + 1], BF16)
                nc.gpsimd.memset(xn[:, :, :, IN_C:IN_C + 1], 1.0)
                state["xn"] = xn
            # node(p, h) = n0 + 4*p + h -> per-partition (h, c) is a 1KB
            # contiguous run; one DMA per macro keeps the APs at 3 dims
            nc.sync.dma_start(
                state["xn"][:, (n0 % DMAT) // MACRO, :, 0:IN_C],
                x_nat[n0:n0 + MACRO, :].rearrange("(p h) c -> p h c", h=4),
            )
            if n0 % TPOSE == 0:
                # host-pretransposed x: 4KB contiguous per partition line
                xt = xt_pool.tile([IN_C, TPOSE], BF16)
                nc.sync.dma_start(xt[:], xT_d[:, n0:n0 + TPOSE])
                state["xt"] = xt
            tiles[m] = {"xn": state["xn"], "xt": state["xt"],
                        "mt": n0 % TPOSE, "mj": (n0 % DMAT) // MACRO}

        def stage_a_mm(m):
            t = tiles[m]
            ph = ph_pool.tile([HID, MACRO], F32)
            nc.tensor.matmul(ph[:], w1_s[:], t["xt"][:, t["mt"]:t["mt"] + MACRO],
                             start=True, stop=True)
            hb = hb_pool.tile([HID, MACRO], BF16)
            nc.scalar.activation(hb[:], ph[:],
                                 mybir.ActivationFunctionType.Tanh, bias=b1_s[:])
            t["hb"] = hb

        def stage_b(m):
            t = tiles[m]
            ps = ps_pool.tile([CHUNK, JPM], F32)
            hbr = t["hb"][:].rearrange("k (p h) -> k h p", h=4)
            for j in range(JPM):
                nc.tensor.matmul(ps[:, j:j + 1],
                                 hbr[:, j, :], w2_s[:],
                                 start=True, stop=True)
            e4 = e4_pool.tile([CHUNK, JPM], F32)
            nc.scalar.activation(e4[:], ps[:],
                                 mybir.ActivationFunctionType.Exp, bias=b2_s[:])
            t["e4"] = e4

        def stage_c1(m):
            t = tiles[m]
            ohes = []
            for j in range(JPM):
                q = m * JPM + j
                ohe = ohe_pool.tile([CHUNK, GPC], BF16)
                nc.vector.tensor_scalar(
                    ohe[:], iota_s[:],
                    gid_s[:, q:q + 1], t["e4"][:, j:j + 1],
                    mybir.AluOpType.is_equal, mybir.AluOpType.mult)
                ohes.append(ohe)
            t["ohes"] = ohes

        def stage_c2(m):
            t = tiles[m]
            for j in range(JPM):
                ci = state["ci"]
                nc.tensor.matmul(pacc[:], t["ohes"][j][:],
                                 t["xn"][:, t["mj"], j, :],
                                 start=(ci == 0), stop=(ci == n_chunks_total - 1))
                state["ci"] = ci + 1
            del tiles[m]

        for m in range(n_macros + 3):
            if m < n_macros:
                stage_a_dma(m)
            if 1 <= m <= n_macros:
                stage_b(m - 1)
            if 2 <= m <= n_macros + 1:
                stage_c1(m - 2)
            if m >= 3:
                stage_c2(m - 3)
            # mlp last: it depends on the freshest DMA; keeping it behind
            # ready work prevents it from blocking the in-order PE queue
            if m < n_macros:
                stage_a_mm(m)

        # ---- final: normalize, transform, write out ----
        rden = fin_pool.tile([GPC, 1], F32, tag="rden")
        nc.vector.reciprocal(rden[:], pacc[:, IN_C:IN_C + 1])
        pooln = fin_pool.tile([GPC, IN_C], F32, tag="pooln")
        nc.vector.tensor_scalar(pooln[:], pacc[:, 0:IN_C], rden[:], None,
                                mybir.AluOpType.mult)
        ptr = pfin_pool.tile([IN_C, GPC], F32, tag="ptr")
        nc.tensor.transpose(ptr[:], pooln[:], id_s[:])
        poolT = fin_pool.tile([IN_C, GPC], F32, tag="poolT")
        nc.scalar.copy(poolT[:], ptr[:])
        pfin = pfin_pool.tile([OUT_C, GPC], F32, tag="pfin")
        nc.tensor.matmul(pfin[:], wt_s[:], poolT[:], start=True, stop=True)
        outT_s = fin_pool.tile([OUT_C, GPC], F32, tag="outT")
        nc.scalar.activation(outT_s[:], pfin[:],
                             mybir.ActivationFunctionType.Identity, bias=bt_s[:])
        nc.sync.dma_start(out_d[:], outT_s[:])

    nc.compile()
    return nc


def kernel(x, batch, W1, b1, W2, b2, Wt, bt, _trace=False, _trace_kwargs=None):
    x = np.asarray(x)
    batch = np.asarray(batch)
    W1 = np.asarray(W1, dtype=np.float32)
    b1 = np.asarray(b1, dtype=np.float32)
    W2 = np.asarray(W2, dtype=np.float32)
    b2 = np.asarray(b2, dtype=np.float32)
    Wt = np.asarray(Wt, dtype=np.float32)
    bt = np.asarray(bt, dtype=np.float32)

    starts = np.searchsorted(batch, np.arange(N_CORES + 1) * GPC).astype(np.int64)
    counts = np.diff(starts)
    npad = int(-(-counts.max() // DMAT) * DMAT)
    nm = npad // CHUNK

    key = npad
    if key not in _CACHE:
        _CACHE[key] = _build(npad)
    nc = _CACHE[key]

    bf16 = ml_dtypes.bfloat16
    iota = np.broadcast_to(np.arange(GPC, dtype=np.float32), (CHUNK, GPC))
    common = {
        "iota": iota.astype(bf16),
        "w1": W1.astype(bf16),
        "b1": b1.reshape(HID, 1).astype(np.float32),
        "w2": W2.reshape(HID, 1).astype(bf16),
        "b2": np.full((CHUNK, 1), float(b2.ravel()[0]), dtype=np.float32),
        "wt": Wt.astype(np.float32),
        "bt": bt.reshape(OUT_C, 1).astype(np.float32),
        "idm": np.eye(128, dtype=np.float32),
    }
    in_maps = []
    for k in range(N_CORES):
        s, e = int(starts[k]), int(starts[k + 1])
        cnt = e - s
        x_nat = np.zeros((npad, IN_C), dtype=bf16)
        x_nat[:cnt] = x[s:e].astype(bf16)
        xT = np.ascontiguousarray(x_nat.T)
        gid_lin = np.full(npad, -1.0, dtype=np.float32)
        gid_lin[:cnt] = (batch[s:e] - k * GPC).astype(np.float32)
        # chunk (G2048, t, h) holds nodes 2048*G + 512*t + 4*p + h
        gid = np.ascontiguousarray(
            gid_lin.reshape(-1, 4, CHUNK, 4).transpose(2, 0, 1, 3).reshape(CHUNK, nm))
        in_maps.append({"x_nat": x_nat, "xT": xT, "gid": gid, **common})

    res = run_bass_kernel_spmd(
        nc, in_maps, core_ids=list(range(N_CORES)),
        trace=_trace, **(_trace_kwargs or {}))

    out = np.empty((G, OUT_C), dtype=np.float32)
    for k in range(N_CORES):
        out[k * GPC:(k + 1) * GPC, :] = res.results[k]["outT"].T
    if _trace:
        return out, res
    return out


# revision 16
# speedup vs baseline: 1.5385x; 1.2028x over previous
"""AttentionReadout kernel for 8 Trainium2 NeuronCores.

Math (per graph g): pooled[g] = sum_i attn_i * x_i with
  attn_i = e_i / sum_{j in g} e_j,  e_i = exp(tanh(x_i @ W1 + b1) @ W2 + b2)
  out = pooled @ Wt + bt

Sharding: graph-aligned data parallel. Core k owns graphs [128k, 128k+128)
and exactly the (contiguous, since batch is sorted) nodes of those graphs.
Each core computes its own 128 graphs end-to-end; no collectives. Host
concatenates the 8 [128, 128] output shards.

Device pipeline (software-pipelined, lag 2 between stages so the PE always
has ready work):
  stage A (macro m):  DMA x natural [n,c] + DMA-transpose [c,n] (bf16),
                      PE: hT[64,512] = W1b.T @ xT;  ACT: h = tanh(hT+b1)
  stage B (macro m-1): PE per 128-chunk: scores[n,1] = h_chunk.T @ W2;
                      ACT: e[128,4] = exp(scores + b2) (bf16)
  stage C (macro m-2): DVE/GpSimd per chunk: ohe[n,g] = (iota==gid)*e;
                      PE per chunk: pacc[g,0:129] += ohe.T @ [x | 1]
                      (column 128 of the rhs is constant 1 -> accumulates
                      the softmax denominator for free)
Final: den=pacc[:,128]; pooled_n = pacc[:,0:128]/den (DVE); PE transpose;
outT[o,g] = Wt.T @ pooled_n.T + bt; DMA out.
"""

import numpy as np
import ml_dtypes
from contextlib import ExitStack

import concourse.bass as bass
import concourse.bacc as bacc
import concourse.tile as tile
from concourse import mybir
from concourse.bass_utils import run_bass_kernel_spmd

N_CORES = 8
G = 1024
GPC = G // N_CORES  # 128 graphs per core
IN_C = 128
HID = 64
OUT_C = 128
MACRO = 512          # nodes per macro tile
DMAT = 2048          # nodes per natural-load DMA tile
TPOSE = 2048         # nodes per DMA-transpose tile
CHUNK = 128          # nodes per chunk (PE contraction width)
JPM = MACRO // CHUNK  # chunks per macro
BF16 = mybir.dt.bfloat16
F32 = mybir.dt.float32

_CACHE = {}


def _build(npad):
    nm = npad // CHUNK        # gid columns
    n_macros = npad // MACRO

    nc = bacc.Bacc("TRN2", target_bir_lowering=False, debug=False,
                   num_devices=N_CORES)

    x_nat = nc.dram_tensor("x_nat", [npad, IN_C], BF16, kind="ExternalInput").ap()
    xT_d = nc.dram_tensor("xT", [IN_C, npad], BF16, kind="ExternalInput").ap()
    gid_d = nc.dram_tensor("gid", [CHUNK, nm], F32, kind="ExternalInput").ap()
    iota_d = nc.dram_tensor("iota", [CHUNK, GPC], BF16, kind="ExternalInput").ap()
    w1_d = nc.dram_tensor("w1", [IN_C, HID], BF16, kind="ExternalInput").ap()
    b1_d = nc.dram_tensor("b1", [HID, 1], F32, kind="ExternalInput").ap()
    w2_d = nc.dram_tensor("w2", [HID, 1], BF16, kind="ExternalInput").ap()
    b2_d = nc.dram_tensor("b2", [CHUNK, 1], F32, kind="ExternalInput").ap()
    wt_d = nc.dram_tensor("wt", [IN_C, OUT_C], F32, kind="ExternalInput").ap()
    bt_d = nc.dram_tensor("bt", [OUT_C, 1], F32, kind="ExternalInput").ap()
    id_d = nc.dram_tensor("idm", [128, 128], F32, kind="ExternalInput").ap()
    out_d = nc.dram_tensor("outT", [OUT_C, GPC], F32, kind="ExternalOutput").ap()

    with tile.TileContext(nc) as tc, ExitStack() as ctx:
        consts = ctx.enter_context(tc.tile_pool(name="consts", bufs=1))
        xn_pool = ctx.enter_context(tc.tile_pool(name="xn", bufs=4))
        xt_pool = ctx.enter_context(tc.tile_pool(name="xt", bufs=4))
        hb_pool = ctx.enter_context(tc.tile_pool(name="hb", bufs=4))
        e4_pool = ctx.enter_context(tc.tile_pool(name="e4", bufs=6))
        ohe_pool = ctx.enter_context(tc.tile_pool(name="ohe", bufs=12))
        fin_pool = ctx.enter_context(tc.tile_pool(name="fin", bufs=1))
        ph_pool = ctx.enter_context(tc.tile_pool(name="ph", bufs=2, space="PSUM"))
        ps_pool = ctx.enter_context(tc.tile_pool(name="ps", bufs=2, space="PSUM"))
        pacc_pool = ctx.enter_context(tc.tile_pool(name="pacc", bufs=1, space="PSUM"))
        pfin_pool = ctx.enter_context(tc.tile_pool(name="pfin", bufs=1, space="PSUM"))

        # constants
        gid_s = consts.tile([CHUNK, nm], F32)
        nc.sync.dma_start(gid_s[:], gid_d[:])
        iota_s = consts.tile([CHUNK, GPC], BF16)
        nc.sync.dma_start(iota_s[:], iota_d[:])
        w1_s = consts.tile([IN_C, HID], BF16)
        nc.sync.dma_start(w1_s[:], w1_d[:])
        b1_s = consts.tile([HID, 1], F32)
        nc.sync.dma_start(b1_s[:], b1_d[:])
        w2_s = consts.tile([HID, 1], BF16)
        nc.sync.dma_start(w2_s[:], w2_d[:])
        b2_s = consts.tile([CHUNK, 1], F32)
        nc.sync.dma_start(b2_s[:], b2_d[:])
        wt_s = consts.tile([IN_C, OUT_C], F32)
        nc.sync.dma_start(wt_s[:], wt_d[:])
        bt_s = consts.tile([OUT_C, 1], F32)
        nc.sync.dma_start(bt_s[:], bt_d[:])
        id_s = consts.tile([128, 128], F32)
        nc.sync.dma_start(id_s[:], id_d[:])

        pacc = [pacc_pool.tile([GPC, IN_C + 1], F32, tag=f"pacc{i}", name=f"pacc{i}")
                for i in range(2)]  # two banks so consecutive accumulating
                                    # matmuls pipeline instead of draining

        n_chunks_total = n_macros * JPM
        tiles = {}  # m -> dict of stage tiles
        state = {"xn": None, "xt": None, "ci": 0}

        def stage_a_dma(m):
            n0 = m * MACRO
            if n0 % DMAT == 0:
                xn = xn_pool.tile([CHUNK, DMAT // MACRO, 4, IN_C + 1], BF16)
                nc.gpsimd.memset(xn[:, :, :, IN_C:IN_C + 1], 1.0)
                state["xn"] = xn
            # node(p, h) = n0 + 4*p + h -> per-partition (h, c) is a 1KB
            # contiguous run; one DMA per macro keeps the APs at 3 dims
            nc.sync.dma_start(
                state["xn"][:, (n0 % DMAT) // MACRO, :, 0:IN_C],
                x_nat[n0:n0 + MACRO, :].rearrange("(p h) c -> p h c", h=4),
            )
            if n0 % TPOSE == 0:
                # host-pretransposed x: 4KB contiguous per partition line
                xt = xt_pool.tile([IN_C, TPOSE], BF16)
                nc.sync.dma_start(xt[:], xT_d[:, n0:n0 + TPOSE])
                state["xt"] = xt
            tiles[m] = {"xn": state["xn"], "xt": state["xt"],
                        "mt": n0 % TPOSE, "mj": (n0 % DMAT) // MACRO}

        def stage_a_mm(m):
            t = tiles[m]
            ph = ph_pool.tile([HID, MACRO], F32)
            nc.tensor.matmul(ph[:], w1_s[:], t["xt"][:, t["mt"]:t["mt"] + MACRO],
                             start=True, stop=True)
            hb = hb_pool.tile([HID, MACRO], BF16)
            nc.scalar.activation(hb[:], ph[:],
                                 mybir.ActivationFunctionType.Tanh, bias=b1_s[:])
            t["hb"] = hb

        def stage_b(m):
            t = tiles[m]
            psk = [ps_pool.tile([CHUNK, 2], F32, tag="ps", name="ps") for _ in range(2)]
            hbr = t["hb"][:].rearrange("k (p h) -> k h p", h=4)
            for j in [0, 2, 1, 3]:  # alternate psum banks -> MMs pipeline
                nc.tensor.matmul(psk[j % 2][:, j // 2:j // 2 + 1],
                                 hbr[:, j, :], w2_s[:],
                                 start=True, stop=True)
            e4s = []
            for i in range(2):
                e4 = e4_pool.tile([CHUNK, 2], F32, tag="e4", name="e4")
                nc.scalar.activation(e4[:], psk[i][:],
                                     mybir.ActivationFunctionType.Exp,
                                     bias=b2_s[:])
                e4s.append(e4)
            t["e4"] = e4s

        def stage_c1(m):
            t = tiles[m]
            ohes = []
            for j in range(JPM):
                q = m * JPM + j
                ohe = ohe_pool.tile([CHUNK, GPC], BF16)
                nc.vector.tensor_scalar(
                    ohe[:], iota_s[:],
                    gid_s[:, q:q + 1], t["e4"][j % 2][:, j // 2:j // 2 + 1],
                    mybir.AluOpType.is_equal, mybir.AluOpType.mult)
                ohes.append(ohe)
            t["ohes"] = ohes

        def stage_c2(m):
            t = tiles[m]
            for j in range(JPM):
                ci = state["ci"]
                nc.tensor.matmul(pacc[ci % 2][:], t["ohes"][j][:],
                                 t["xn"][:, t["mj"], j, :],
                                 start=(ci < 2), stop=(ci >= n_chunks_total - 2))
                state["ci"] = ci + 1
            del tiles[m]

        for m in range(n_macros + 3):
            if m < n_macros:
                stage_a_dma(m)
            if 1 <= m <= n_macros:
                stage_b(m - 1)
            if 2 <= m <= n_macros + 1:
                stage_c1(m - 2)
            if m >= 3:
                stage_c2(m - 3)
            # mlp last: it depends on the freshest DMA; keeping it behind
            # ready work prevents it from blocking the in-order PE queue
            if m < n_macros:
                stage_a_mm(m)

        # ---- final: normalize, transform, write out ----
        pacc0_sb = fin_pool.tile([GPC, IN_C + 1], F32, tag="pacc0_sb")
        nc.scalar.copy(pacc0_sb[:], pacc[0][:])
        psum2 = fin_pool.tile([GPC, IN_C + 1], F32, tag="psum2")
        nc.vector.tensor_tensor(psum2[:], pacc0_sb[:], pacc[1][:],
                                mybir.AluOpType.add)
        rden = fin_pool.tile([GPC, 1], F32, tag="rden")
        nc.vector.reciprocal(rden[:], psum2[:, IN_C:IN_C + 1])
        pooln = fin_pool.tile([GPC, IN_C], F32, tag="pooln")
        nc.vector.tensor_scalar(pooln[:], psum2[:, 0:IN_C], rden[:], None,
                                mybir.AluOpType.mult)
        ptr = pfin_pool.tile([IN_C, GPC], F32, tag="ptr")
        nc.tensor.transpose(ptr[:], pooln[:], id_s[:])
        poolT = fin_pool.tile([IN_C, GPC], F32, tag="poolT")
        nc.scalar.copy(poolT[:], ptr[:])
        pfin = pfin_pool.tile([OUT_C, GPC], F32, tag="pfin")
        nc.tensor.matmul(pfin[:], wt_s[:], poolT[:], start=True, stop=True)
        outT_s = fin_pool.tile([OUT_C, GPC], F32, tag="outT")
        nc.scalar.activation(outT_s[:], pfin[:],
                             mybir.ActivationFunctionType.Identity, bias=bt_s[:])
        nc.sync.dma_start(out_d[:], outT_s[:])

    nc.compile()
    return nc


def kernel(x, batch, W1, b1, W2, b2, Wt, bt, _trace=False, _trace_kwargs=None):
    x = np.asarray(x)
    batch = np.asarray(batch)
    W1 = np.asarray(W1, dtype=np.float32)
    b1 = np.asarray(b1, dtype=np.float32)
    W2 = np.asarray(W2, dtype=np.float32)
    b2 = np.asarray(b2, dtype=np.float32)
    Wt = np.asarray(Wt, dtype=np.float32)
    bt = np.asarray(bt, dtype=np.float32)

    starts = np.searchsorted(batch, np.arange(N_CORES + 1) * GPC).astype(np.int64)
    counts = np.diff(starts)
    npad = int(-(-counts.max() // DMAT) * DMAT)
    nm = npad // CHUNK

    key = npad
    if key not in _CACHE:
        _CACHE[key] = _build(npad)
    nc = _CACHE[key]

    bf16 = ml_dtypes.bfloat16
    iota = np.broadcast_to(np.arange(GPC, dtype=np.float32), (CHUNK, GPC))
    common = {
        "iota": iota.astype(bf16),
        "w1": W1.astype(bf16),
        "b1": b1.reshape(HID, 1).astype(np.float32),
        "w2": W2.reshape(HID, 1).astype(bf16),
        "b2": np.full((CHUNK, 1), float(b2.ravel()[0]), dtype=np.float32),
        "wt": Wt.astype(np.float32),
        "bt": bt.reshape(OUT_C, 1).astype(np.float32),
        "idm": np.eye(128, dtype=np.float32),
    }
    in_maps = []
    for k in range(N_CORES):
        s, e = int(starts[k]), int(starts[k + 1])
        cnt = e - s
        x_nat = np.zeros((npad, IN_C), dtype=bf16)
        x_nat[:cnt] = x[s:e].astype(bf16)
        xT = np.ascontiguousarray(x_nat.T)
        gid_lin = np.full(npad, -1.0, dtype=np.float32)
        gid_lin[:cnt] = (batch[s:e] - k * GPC).astype(np.float32)
        # chunk (G2048, t, h) holds nodes 2048*G + 512*t + 4*p + h
        gid = np.ascontiguousarray(
            gid_lin.reshape(-1, 4, CHUNK, 4).transpose(2, 0, 1, 3).reshape(CHUNK, nm))
        in_maps.append({"x_nat": x_nat, "xT": xT, "gid": gid, **common})

    res = run_bass_kernel_spmd(
        nc, in_maps, core_ids=list(range(N_CORES)),
        trace=_trace, **(_trace_kwargs or {}))

    out = np.empty((G, OUT_C), dtype=np.float32)
    for k in range(N_CORES):
        out[k * GPC:(k + 1) * GPC, :] = res.results[k]["outT"].T
    if _trace:
        return out, res
    return out


# revision 21
# speedup vs baseline: 1.7686x; 1.1496x over previous
"""AttentionReadout kernel for 8 Trainium2 NeuronCores.

Math (per graph g): pooled[g] = sum_i attn_i * x_i with
  attn_i = e_i / sum_{j in g} e_j,  e_i = exp(tanh(x_i @ W1 + b1) @ W2 + b2)
  out = pooled @ Wt + bt

Sharding: graph-aligned data parallel. Core k owns graphs [128k, 128k+128)
and exactly the (contiguous, since batch is sorted) nodes of those graphs.
Each core computes its own 128 graphs end-to-end; no collectives. Host
concatenates the 8 [128, 128] output shards.

Device pipeline (software-pipelined, lag 2 between stages so the PE always
has ready work):
  stage A (macro m):  DMA x natural [n,c] + DMA-transpose [c,n] (bf16),
                      PE: hT[64,512] = W1b.T @ xT;  ACT: h = tanh(hT+b1)
  stage B (macro m-1): PE per 128-chunk: scores[n,1] = h_chunk.T @ W2;
                      ACT: e[128,4] = exp(scores + b2) (bf16)
  stage C (macro m-2): DVE/GpSimd per chunk: ohe[n,g] = (iota==gid)*e;
                      PE per chunk: pacc[g,0:129] += ohe.T @ [x | 1]
                      (column 128 of the rhs is constant 1 -> accumulates
                      the softmax denominator for free)
Final: den=pacc[:,128]; pooled_n = pacc[:,0:128]/den (DVE); PE transpose;
outT[o,g] = Wt.T @ pooled_n.T + bt; DMA out.
"""

import numpy as np
import ml_dtypes
from contextlib import ExitStack

import concourse.bass as bass
import concourse.bacc as bacc
import concourse.tile as tile
from concourse import mybir
from concourse.bass_utils import run_bass_kernel_spmd

N_CORES = 8
G = 1024
GPC = G // N_CORES  # 128 graphs per core
IN_C = 128
HID = 64
OUT_C = 128
MACRO = 512          # nodes per macro tile
DMAT = 2048          # nodes per natural-load DMA tile
TPOSE = 2048         # nodes per DMA-transpose tile
CHUNK = 128          # nodes per chunk (PE contraction width)
JPM = MACRO // CHUNK  # chunks per macro
BF16 = mybir.dt.bfloat16
F32 = mybir.dt.float32

_CACHE = {}


def _build(npad):
    nm = npad // CHUNK        # gid columns
    n_macros = npad // MACRO

    nc = bacc.Bacc("TRN2", target_bir_lowering=False, debug=False,
                   num_devices=N_CORES)

    x_nat = nc.dram_tensor("x_nat", [npad, IN_C], BF16, kind="ExternalInput").ap()
    xT_d = nc.dram_tensor("xT", [IN_C, npad], BF16, kind="ExternalInput").ap()
    gid_d = nc.dram_tensor("gid", [CHUNK, nm], BF16, kind="ExternalInput").ap()
    iota_d = nc.dram_tensor("iota", [CHUNK, GPC], BF16, kind="ExternalInput").ap()
    w1_d = nc.dram_tensor("w1", [IN_C, HID], BF16, kind="ExternalInput").ap()
    b1_d = nc.dram_tensor("b1", [2 * HID, 1], F32, kind="ExternalInput").ap()
    w2_d = nc.dram_tensor("w2", [2 * HID, 1], BF16, kind="ExternalInput").ap()
    b2_d = nc.dram_tensor("b2", [CHUNK, 1], F32, kind="ExternalInput").ap()
    wt_d = nc.dram_tensor("wt", [IN_C, OUT_C], F32, kind="ExternalInput").ap()
    bt_d = nc.dram_tensor("bt", [OUT_C, 1], F32, kind="ExternalInput").ap()
    id_d = nc.dram_tensor("idm", [128, 128], F32, kind="ExternalInput").ap()
    out_d = nc.dram_tensor("outT", [OUT_C, GPC], F32, kind="ExternalOutput").ap()

    with tile.TileContext(nc) as tc, ExitStack() as ctx:
        consts = ctx.enter_context(tc.tile_pool(name="consts", bufs=1))
        xn_pool = ctx.enter_context(tc.tile_pool(name="xn", bufs=4))
        xt_pool = ctx.enter_context(tc.tile_pool(name="xt", bufs=4))
        hb_pool = ctx.enter_context(tc.tile_pool(name="hb", bufs=4))
        e_pool = ctx.enter_context(tc.tile_pool(name="e", bufs=3))
        oh_pool = ctx.enter_context(tc.tile_pool(name="oh", bufs=3))
        z_pool = ctx.enter_context(tc.tile_pool(name="z", bufs=3))
        fin_pool = ctx.enter_context(tc.tile_pool(name="fin", bufs=1))
        ph_pool = ctx.enter_context(tc.tile_pool(name="ph", bufs=2, space="PSUM"))
        ps_pool = ctx.enter_context(tc.tile_pool(name="ps", bufs=2, space="PSUM"))
        pacc_pool = ctx.enter_context(tc.tile_pool(name="pacc", bufs=1, space="PSUM"))
        pfin_pool = ctx.enter_context(tc.tile_pool(name="pfin", bufs=1, space="PSUM"))

        # constants
        gid_s = consts.tile([CHUNK, nm], BF16)
        nc.sync.dma_start(gid_s[:], gid_d[:])
        iota_s = consts.tile([CHUNK, GPC], BF16)
        nc.sync.dma_start(iota_s[:], iota_d[:])
        w1_s = consts.tile([IN_C, HID], BF16)
        nc.sync.dma_start(w1_s[:], w1_d[:])
        b1_s = consts.tile([2 * HID, 1], F32)
        nc.sync.dma_start(b1_s[:], b1_d[:])
        w2_s = consts.tile([2 * HID, 1], BF16)
        nc.sync.dma_start(w2_s[:], w2_d[:])
        b2_s = consts.tile([CHUNK, 1], F32)
        nc.sync.dma_start(b2_s[:], b2_d[:])
        wt_s = consts.tile([IN_C, OUT_C], F32)
        nc.sync.dma_start(wt_s[:], wt_d[:])
        bt_s = consts.tile([OUT_C, 1], F32)
        nc.sync.dma_start(bt_s[:], bt_d[:])
        id_s = consts.tile([128, 128], F32)
        nc.sync.dma_start(id_s[:], id_d[:])

        pacc = [pacc_pool.tile([GPC, IN_C + 1], F32, tag=f"pacc{i}", name=f"pacc{i}")
                for i in range(2)]  # two banks so consecutive accumulating
                                    # matmuls pipeline instead of draining

        n_groups = npad // DMAT
        CPG = DMAT // CHUNK            # 16 chunks per group
        n_chunks_total = n_groups * CPG
        tiles = {}
        state = {"ci": 0}

        def stage_a(g):
            n0 = g * DMAT
            xn = xn_pool.tile([CHUNK, CPG, IN_C + 1], BF16)
            nc.gpsimd.memset(xn[:, :, IN_C:IN_C + 1], 1.0)
            for mm in range(DMAT // MACRO):
                # node(p, h) = n0 + 512*mm + 4*p + h: 1KB contiguous runs
                nc.sync.dma_start(
                    xn[:, mm * 4:(mm + 1) * 4, 0:IN_C],
                    x_nat[n0 + mm * MACRO:n0 + (mm + 1) * MACRO, :]
                    .rearrange("(p h) c -> p h c", h=4),
                )
            xt = xt_pool.tile([IN_C, DMAT], BF16)
            nc.sync.dma_start(xt[:], xT_d[:, n0:n0 + DMAT])
            tiles[g] = {"xn": xn, "xt": xt}

        def stage_mlp(g):
            t = tiles[g]
            hbs = []
            for mp in range(DMAT // (2 * MACRO)):   # 2 macro-pairs
                ph2 = ph_pool.tile([2 * HID, MACRO], F32)
                nc.tensor.matmul(ph2[0:HID, :], w1_s[:],
                                 t["xt"][:, (2 * mp) * MACRO:(2 * mp + 1) * MACRO],
                                 start=True, stop=True)
                nc.tensor.matmul(ph2[HID:2 * HID, :], w1_s[:],
                                 t["xt"][:, (2 * mp + 1) * MACRO:(2 * mp + 2) * MACRO],
                                 start=True, stop=True, tile_position=(0, HID))
                hb2 = hb_pool.tile([2 * HID, MACRO], BF16)
                nc.scalar.activation(hb2[:], ph2[:],
                                     mybir.ActivationFunctionType.Tanh,
                                     bias=b1_s[:])
                hbs.append(hb2)
            t["hbs"] = hbs

        def stage_b(g):
            t = tiles[g]
            psk = [ps_pool.tile([CHUNK, CPG // 2], F32, tag="ps", name="ps")
                   for _ in range(2)]
            for qq in range(CPG):                   # chunk qq = (m, j)
                m, j = qq // 4, qq % 4
                mp, s = m // 2, m % 2
                hbr = t["hbs"][mp][:].rearrange("k (p h) -> k h p", h=4)
                nc.tensor.matmul(psk[qq % 2][:, qq // 2:qq // 2 + 1],
                                 hbr[s * HID:(s + 1) * HID, j, :],
                                 w2_s[s * HID:(s + 1) * HID, :],
                                 start=True, stop=True)
            egrp = e_pool.tile([CHUNK, CPG], BF16)
            for i in range(2):
                nc.scalar.activation(egrp[:, i:CPG:2], psk[i][:],
                                     mybir.ActivationFunctionType.Exp,
                                     bias=b2_s[:])
            t["egrp"] = egrp

        def stage_c1(g):
            t = tiles[g]
            q0 = g * CPG
            oh = oh_pool.tile([CHUNK, CPG, GPC], BF16)
            nc.vector.tensor_tensor(
                oh[:],
                iota_s[:].rearrange("p (q g) -> p q g", q=1).broadcast_to([CHUNK, CPG, GPC]),
                gid_s[:, q0:q0 + CPG].rearrange("p (q g) -> p q g", g=1)
                .broadcast_to([CHUNK, CPG, GPC]),
                mybir.AluOpType.is_equal)
            z = z_pool.tile([CHUNK, CPG, IN_C + 1], BF16)
            nc.vector.tensor_tensor(
                z[:], t["xn"][:],
                t["egrp"][:].rearrange("p (q c) -> p q c", c=1)
                .broadcast_to([CHUNK, CPG, IN_C + 1]),
                mybir.AluOpType.mult)
            t["oh"] = oh
            t["z"] = z

        def stage_c2(g):
            t = tiles[g]
            for qq in range(CPG):
                ci = state["ci"]
                nc.tensor.matmul(pacc[ci % 2][:], t["oh"][:, qq, :],
                                 t["z"][:, qq, :],
                                 start=(ci < 2), stop=(ci >= n_chunks_total - 2))
                state["ci"] = ci + 1
            del tiles[g]

        for g in range(n_groups + 2):
            if g < n_groups:
                stage_a(g)
            if g >= 2:
                stage_c2(g - 2)
            if 1 <= g <= n_groups:
                stage_b(g - 1)
                stage_c1(g - 1)
            if g < n_groups:
                stage_mlp(g)

        # ---- final: normalize, transform, write out ----
        pacc0_sb = fin_pool.tile([GPC, IN_C + 1], F32, tag="pacc0_sb")
        nc.scalar.copy(pacc0_sb[:], pacc[0][:])
        psum2 = fin_pool.tile([GPC, IN_C + 1], F32, tag="psum2")
        nc.vector.tensor_tensor(psum2[:], pacc0_sb[:], pacc[1][:],
                                mybir.AluOpType.add)
        rden = fin_pool.tile([GPC, 1], F32, tag="rden")
        nc.vector.reciprocal(rden[:], psum2[:, IN_C:IN_C + 1])
        pooln = fin_pool.tile([GPC, IN_C], F32, tag="pooln")
        nc.vector.tensor_scalar(pooln[:], psum2[:, 0:IN_C], rden[:], None,
                                mybir.AluOpType.mult)
        ptr = pfin_pool.tile([IN_C, GPC], F32, tag="ptr")
        nc.tensor.transpose(ptr[:], pooln[:], id_s[:])
        poolT = fin_pool.tile([IN_C, GPC], F32, tag="poolT")
        nc.scalar.copy(poolT[:], ptr[:])
        pfin = pfin_pool.tile([OUT_C, GPC], F32, tag="pfin")
        nc.tensor.matmul(pfin[:], wt_s[:], poolT[:], start=True, stop=True)
        outT_s = fin_pool.tile([OUT_C, GPC], F32, tag="outT")
        nc.scalar.activation(outT_s[:], pfin[:],
                             mybir.ActivationFunctionType.Identity, bias=bt_s[:])
        nc.sync.dma_start(out_d[:], outT_s[:])

    nc.compile()
    return nc


def kernel(x, batch, W1, b1, W2, b2, Wt, bt, _trace=False, _trace_kwargs=None):
    x = np.asarray(x)
    batch = np.asarray(batch)
    W1 = np.asarray(W1, dtype=np.float32)
    b1 = np.asarray(b1, dtype=np.float32)
    W2 = np.asarray(W2, dtype=np.float32)
    b2 = np.asarray(b2, dtype=np.float32)
    Wt = np.asarray(Wt, dtype=np.float32)
    bt = np.asarray(bt, dtype=np.float32)

    starts = np.searchsorted(batch, np.arange(N_CORES + 1) * GPC).astype(np.int64)
    counts = np.diff(starts)
    npad = int(-(-counts.max() // DMAT) * DMAT)
    nm = npad // CHUNK

    key = npad
    if key not in _CACHE:
        _CACHE[key] = _build(npad)
    nc = _CACHE[key]

    bf16 = ml_dtypes.bfloat16
    iota = np.broadcast_to(np.arange(GPC, dtype=np.float32), (CHUNK, GPC))
    common = {
        "iota": iota.astype(bf16),
        "w1": W1.astype(bf16),
        "b1": np.tile(b1.reshape(HID, 1), (2, 1)).astype(np.float32),
        "w2": np.tile(W2.reshape(HID, 1), (2, 1)).astype(bf16),
        "b2": np.full((CHUNK, 1), float(b2.ravel()[0]), dtype=np.float32),
        "wt": Wt.astype(np.float32),
        "bt": bt.reshape(OUT_C, 1).astype(np.float32),
        "idm": np.eye(128, dtype=np.float32),
    }
    in_maps = []
    for k in range(N_CORES):
        s, e = int(starts[k]), int(starts[k + 1])
        cnt = e - s
        x_nat = np.zeros((npad, IN_C), dtype=bf16)
        x_nat[:cnt] = x[s:e].astype(bf16)
        xT = np.ascontiguousarray(x_nat.T)
        gid_lin = np.full(npad, -1.0, dtype=np.float32)
        gid_lin[:cnt] = (batch[s:e] - k * GPC).astype(np.float32)
        # chunk (G2048, t, h) holds nodes 2048*G + 512*t + 4*p + h
        gid = np.ascontiguousarray(
            gid_lin.reshape(-1, 4, CHUNK, 4).transpose(2, 0, 1, 3)
            .reshape(CHUNK, nm)).astype(bf16)
        in_maps.append({"x_nat": x_nat, "xT": xT, "gid": gid, **common})

    res = run_bass_kernel_spmd(
        nc, in_maps, core_ids=list(range(N_CORES)),
        trace=_trace, **(_trace_kwargs or {}))

    out = np.empty((G, OUT_C), dtype=np.float32)
    for k in range(N_CORES):
        out[k * GPC:(k + 1) * GPC, :] = res.results[k]["outT"].T
    if _trace:
        return out, res
    return out
